# revision 44
# baseline (speedup 1.0000x reference)
"""Distributed GCNII-style graph convolution on 8 Trainium2 NeuronCores.

reference:
    msgs    = features[edge_src] * edge_vals[:, None]
    hi      = segment_sum(msgs, edge_dst, N)
    support = (1-ALPHA)*hi + ALPHA*features0
    out     = relu(BETA*(support @ W) + (1-BETA)*support)
            = relu(support @ W'),  W' = BETA*W + (1-BETA)*I

sharding: nodes (rows) split across 8 cores by edge_dst.  Within a core,
nodes are bin-packed into tiles of <=TILE=56 nodes such that each
(tile, src%4 residue) class holds <=128 edges; packing is residue-BALANCED
(pick nodes whose dominant class is the least-filled one) so the four
class caps fill evenly -- T drops 280 -> 256 (~9% fewer gather
descriptors and matmul chunks vs sequential packing at TILE=48).  `features` is replicated to every core so the src gather is
local (the "all-gather" happens at input-distribution time).

gather: the HW `dma_gather` instruction takes int16 indices, so the
[100000, 64] f32 table is addressed as 25000 4-row units (stride 1 KB).
Edges in residue class r = src%4 gather 64 f32 at unit src//4 with base
offset r*64 elements.  One dma_gather call per (group-of-8-tiles, residue),
spread across the 4 SWDGE queues (queue_num=r); indices are wrapped 16-wide
and replicated to 128 partitions as the ucode expects.  Each call is 1024
indices = exactly 64 descriptors per SDMA engine, emitted with
single_packet=True so each engine's stream coalesces into one packet
(first/concatenate/last) instead of 64 singleton packets -- fewer DMA
events and less per-packet overhead on the m2s/s2m bus.  (A bf16 table
variant -- 2-node 256B blocks, bf16 A + bf16 matmuls -- was tried and is
12% SLOWER on HW: descriptors stay 256B so gather traffic is unchanged,
and the DVE/PE rate gains do not materialize.)

per-core device program (SPMD, one Bass program):
  - gather G[p, c, :] = features[src[p, c], :]      (POOL dma_gather, 4 queues)
  - A[p, c, n] = 0.9*val[p,c] * (dstcol[p,c] == n)  built FACTORED:
    H = val x onehot7(dst//8), L = onehot8(dst%8) (tiny DVE compares),
    then A = H (x) L via one broadcast outer-product mult -- ~1.4x fewer
    DVE element-ops than a 56-wide compare+mult.  Keep ALL ops on DVE --
    running any on GPSIMD looks good in CoreSim but is 4x slower on HW
    (Q7 tensor ops contend with SWDGE descriptor generation).
  - PSUM[64f, 448n] += G_chunk.T @ A_chunk          (PE, per 448-node group)
  - support_T = PSUM + 0.1*features0_T_slice        (DVE)
  - out_T = relu(W'.T @ support_T)                  (PE + ACT)
  - transposed [feature, node] layout throughout; host untransposes and
    un-permutes the packed node order.
"""

import os
import sys

import numpy as np


def _import_concourse():
    try:
        import concourse  # noqa: F401
    except ImportError:
        for p in ("/opt/trn_rl_repo", "/root/.axon_site/_ro/trn_rl_repo"):
            if os.path.isdir(p) and p not in sys.path:
                sys.path.insert(0, p)
        import concourse  # noqa: F401


# problem constants (hardcoded; harness gives full-size inputs)
N_NODES = 100000
N_EDGES = 1000000
F = 64
ALPHA = 0.1
BETA = 0.5
N_CORES = 8

TILE = 56          # max nodes per tile (A matrix width, 7x8 for the
                   # factored one-hot); class capacity 4*128 edges ~ 51
                   # nodes, so the edge caps bind before the node cap
GROUP_TILES = 8    # tiles per PSUM group -> 448 nodes per group (<=512 f32);
                   # also 8*128 idxs/gather = 64 descs/engine = one full packet
P = 128            # SBUF partitions / edges per chunk
R = 4              # src residue classes (int16 index limit workaround)


def _pack_tiles(deg):
    """Residue-balanced packing: nodes -> tiles with <=TILE nodes and
    <=P edges per residue class.  Nodes are bucketed by dominant residue
    class; each slot picks from the bucket of the least-filled class so
    the four per-class caps fill evenly.  deg: [shard, R] int.  Returns
    (tile_of_node, pos_of_node, ntiles)."""
    shard = deg.shape[0]
    tile_of = np.empty(shard, np.int32)
    pos_of = np.empty(shard, np.int32)
    dom = np.argmax(deg, axis=1)
    buckets = [list(np.nonzero(dom == r)[0][::-1]) for r in range(R)]
    cnt = np.zeros(R, np.int64)
    t, nn, placed = 0, 0, 0
    while placed < shard:
        node = -1
        if nn < TILE:
            for r in np.argsort(cnt):
                if buckets[r]:
                    i = buckets[r][-1]
                    if np.all(cnt + deg[i] <= P):
                        node = i
                        buckets[r].pop()
                        break
        if node < 0:
            if nn == 0:       # single node exceeds a class cap: impossible
                raise AssertionError("node degree exceeds chunk capacity")
            t += 1
            cnt[:] = 0
            nn = 0
            continue
        tile_of[node], pos_of[node] = t, nn
        cnt += deg[node]
        nn += 1
        placed += 1
    return tile_of, pos_of, t + 1


def _prep(features, features0, edge_src, edge_dst, edge_vals, W,
          n_nodes=N_NODES, n_cores=N_CORES):
    """Host-side sharding.  Returns (in_maps, T, node_cols)."""
    f32 = np.float32
    assert n_nodes % R == 0
    shard = n_nodes // n_cores

    core = np.clip(edge_dst // shard, 0, n_cores - 1)
    dst_local = edge_dst - core * shard
    res = edge_src % R

    # per-core greedy tile packing
    tile_of = np.empty(n_nodes, np.int32)
    pos_of = np.empty(n_nodes, np.int32)
    ntiles = []
    for c in range(n_cores):
        deg = np.zeros((shard, R), np.int32)
        m = core == c
        np.add.at(deg, (dst_local[m], res[m]), 1)
        tl, ps, nt = _pack_tiles(deg)
        sl = slice(c * shard, (c + 1) * shard)
        tile_of[sl], pos_of[sl] = tl, ps
        ntiles.append(nt)
    T = ((max(ntiles) + GROUP_TILES - 1) // GROUP_TILES) * GROUP_TILES
    NCHUNK = GROUP_TILES * R
    NCOL = T * R                      # total chunks per core

    etile = tile_of[edge_dst]         # tile of edge's dst (within its core)
    # chunk column: g*NCHUNK + r*GROUP_TILES + t_local
    col = ((etile // GROUP_TILES) * NCHUNK + res * GROUP_TILES
           + etile % GROUP_TILES)
    key = core * NCOL + col
    counts = np.bincount(key, minlength=n_cores * NCOL)
    assert counts.max() <= P, "tile packing violated chunk capacity"
    order = np.argsort(key, kind="stable")
    sk = key[order]
    starts = np.concatenate([[0], np.cumsum(counts)[:-1]])
    part = np.arange(len(sk), dtype=np.int64) - starts[sk]
    col_s = sk % NCOL
    core_s = sk // NCOL

    unit_all = np.zeros((n_cores, P, NCOL), np.int32)
    dsth_all = np.zeros((n_cores, P, NCOL), f32)
    dstl_all = np.zeros((n_cores, P, NCOL), f32)
    val_all = np.zeros((n_cores, P, NCOL), f32)
    pos_s = pos_of[edge_dst[order]]
    unit_all[core_s, part, col_s] = edge_src[order] // R
    dsth_all[core_s, part, col_s] = (pos_s // 8).astype(f32)
    dstl_all[core_s, part, col_s] = (pos_s % 8).astype(f32)
    val_all[core_s, part, col_s] = ((1.0 - ALPHA) * edge_vals[order]
                                    ).astype(f32)

    # idx16: per (group, residue) call covering chunk cols
    # [g*NCHUNK + r*GROUP_TILES, +GROUP_TILES); flat list i = chunk*128+p;
    # ucode reads list element i from partition i%16, column i//16,
    # replicated across the 8 16-row blocks.
    idx16_all = np.zeros((n_cores, P, NCOL * P // 16), np.int16)
    for cidx in range(n_cores):
        blocks = []
        grid = unit_all[cidx]
        for g in range(T // GROUP_TILES):
            for r in range(R):
                c0 = g * NCHUNK + r * GROUP_TILES
                flat = grid[:, c0:c0 + GROUP_TILES].T.ravel()
                blk = flat.reshape(-1, 16).T
                blocks.append(np.tile(blk, (8, 1)))
        idx16_all[cidx] = np.concatenate(blocks, axis=1).astype(np.int16)

    Wp = (BETA * W + (1.0 - BETA) * np.eye(F, dtype=f32)).astype(f32)
    iota6 = np.broadcast_to(np.arange(TILE // 8, dtype=f32),
                            (P, TILE // 8)).copy()
    iota8 = np.broadcast_to(np.arange(8, dtype=f32), (P, 8)).copy()
    feat = np.ascontiguousarray(features, dtype=f32)

    in_maps = []
    node_cols = []                    # per core: output column of each node
    for c in range(n_cores):
        sl = slice(c * shard, (c + 1) * shard)
        cols = tile_of[sl].astype(np.int64) * TILE + pos_of[sl]
        node_cols.append(cols)
        f0sT = np.zeros((F, T * TILE), f32)
        f0sT[:, cols] = (ALPHA * features0[sl]).T
        in_maps.append({
            "features": feat,
            "eidx": np.ascontiguousarray(idx16_all[c]),
            "edsth": np.ascontiguousarray(dsth_all[c]),
            "edstl": np.ascontiguousarray(dstl_all[c]),
            "eval": np.ascontiguousarray(val_all[c]),
            "f0sT": f0sT,
            "Wp": Wp,
            "iota6": iota6,
            "iota8": iota8,
        })
    return in_maps, T, node_cols


def _build(T, n_nodes=N_NODES, passes=1, skip=(),
           gather_elem=F, gather_queues=R, gather_res=R):
    """Build the SPMD Bass/Tile program.  Returns nc (unfinalized)."""
    from contextlib import ExitStack

    from concourse import bacc, mybir, tile
    from concourse.bass import AP

    f32, i16 = mybir.dt.float32, mybir.dt.int16
    NCOL = T * R
    NG = T // GROUP_TILES                        # groups per core
    NCHUNK = GROUP_TILES * R                     # chunks per group
    GN = TILE * GROUP_TILES                      # nodes per group (480)
    WIDTH = T * TILE                             # outT columns
    IDX16 = NCOL * P // 16
    n_units = n_nodes // R

    nc = bacc.Bacc(num_swdge_queues=4)
    H6, L8 = TILE // 8, 8
    feat_d = nc.dram_tensor("features", [n_nodes, F], f32, kind="ExternalInput")
    idx_d = nc.dram_tensor("eidx", [P, IDX16], i16, kind="ExternalInput")
    dsth_d = nc.dram_tensor("edsth", [P, NCOL], f32, kind="ExternalInput")
    dstl_d = nc.dram_tensor("edstl", [P, NCOL], f32, kind="ExternalInput")
    val_d = nc.dram_tensor("eval", [P, NCOL], f32, kind="ExternalInput")
    f0_d = nc.dram_tensor("f0sT", [F, WIDTH], f32, kind="ExternalInput")
    w_d = nc.dram_tensor("Wp", [F, F], f32, kind="ExternalInput")
    iota6_d = nc.dram_tensor("iota6", [P, H6], f32, kind="ExternalInput")
    iota8_d = nc.dram_tensor("iota8", [P, L8], f32, kind="ExternalInput")
    out_d = nc.dram_tensor("outT", [F, WIDTH], f32, kind="ExternalOutput")
    feat_ap = feat_d[:]

    with tile.TileContext(nc) as tc, ExitStack() as ctx:
        import os as _os
        _b = lambda k, d: int(_os.environ.get("KBUFS_" + k, d))
        const = ctx.enter_context(tc.tile_pool(name="const", bufs=1))
        gpool = ctx.enter_context(tc.tile_pool(name="g", bufs=_b("G", 3)))
        apool = ctx.enter_context(tc.tile_pool(name="a", bufs=_b("A", 2)))
        hpool = ctx.enter_context(tc.tile_pool(name="h", bufs=2))
        lpool = ctx.enter_context(tc.tile_pool(name="l", bufs=2))
        spool = ctx.enter_context(tc.tile_pool(name="sup", bufs=_b("S", 2)))
        opool = ctx.enter_context(tc.tile_pool(name="o", bufs=_b("O", 2)))
        pspool = ctx.enter_context(tc.tile_pool(name="ps", bufs=_b("PS", 2),
                                                space="PSUM"))
        ps2pool = ctx.enter_context(tc.tile_pool(name="ps2", bufs=_b("PS2", 2),
                                                 space="PSUM"))

        idx_sb = const.tile([P, IDX16], i16)
        dsth_sb = const.tile([P, NCOL], f32)
        dstl_sb = const.tile([P, NCOL], f32)
        val_sb = const.tile([P, NCOL], f32)
        f0_sb = const.tile([F, WIDTH], f32)
        w_sb = const.tile([F, F], f32)
        iota6_sb = const.tile([P, H6], f32)
        iota8_sb = const.tile([P, L8], f32)
        nc.sync.dma_start(idx_sb[:], idx_d[:])
        nc.sync.dma_start(dsth_sb[:], dsth_d[:])
        nc.sync.dma_start(dstl_sb[:], dstl_d[:])
        nc.sync.dma_start(val_sb[:], val_d[:])
        nc.sync.dma_start(f0_sb[:], f0_d[:])
        nc.sync.dma_start(w_sb[:], w_d[:])
        nc.sync.dma_start(iota6_sb[:], iota6_d[:])
        nc.sync.dma_start(iota8_sb[:], iota8_d[:])

        iota6_ap = iota6_sb[:]
        iota8_ap = iota8_sb[:]
        for _pass in range(passes):
          for g in range(NG):
              col0 = g * NCHUNK
              gt = gpool.tile([P, NCHUNK, F], f32)
              if 'gather' in skip:
                  nc.sync.dma_start(gt[:F, 0, :], f0_d[:, :F])
              else:
                  # one 1024-idx call per (group, residue): 64 descs/engine
                  # = one full coalesced packet.  Splitting into 2x512 costs
                  # ~0.8us/call of Pool overhead (measured +100us/pass);
                  # fewer+bigger calls would need >64-desc packets.
                  for r in range(gather_res):
                      num_idxs = GROUP_TILES * P
                      off16 = (g * NCHUNK + r * GROUP_TILES) * P // 16
                      src_ap = AP(feat_ap.tensor, r * F,
                                  [[R * F, n_units], [1, F]])
                      nc.gpsimd.dma_gather(
                          out_ap=gt[:, r * GROUP_TILES:(r + 1) * GROUP_TILES,
                                    :gather_elem],
                          in_ap=src_ap,
                          idxs_ap=idx_sb[:, off16:off16 + num_idxs // 16],
                          num_idxs=num_idxs,
                          num_idxs_reg=num_idxs,
                          elem_size=gather_elem,
                          elem_step=R * F,
                          single_packet=True,
                          queue_num=r % gather_queues,
                      )

              # A = (val x onehot6(dst//8)) (x) onehot8(dst%8): the compares
              # run on 6- and 8-wide tensors; only the outer-product combine
              # touches the full 48 width -- ~1.4x fewer DVE element-ops
              # than a 48-wide compare + mult.
              at = apool.tile([P, NCHUNK, TILE], f32)
              ht = hpool.tile([P, NCHUNK, H6], f32)
              lt = lpool.tile([P, NCHUNK, L8], f32)
              iota6_bc = AP(iota6_ap.tensor, iota6_ap.offset,
                            [iota6_ap.ap[0], [0, NCHUNK], iota6_ap.ap[1]])
              iota8_bc = AP(iota8_ap.tensor, iota8_ap.offset,
                            [iota8_ap.ap[0], [0, NCHUNK], iota8_ap.ap[1]])
              dsth_bc = dsth_sb[:, col0:col0 + NCHUNK].broadcast_to(
                  [P, NCHUNK, H6])
              dstl_bc = dstl_sb[:, col0:col0 + NCHUNK].broadcast_to(
                  [P, NCHUNK, L8])
              val_bc = val_sb[:, col0:col0 + NCHUNK].broadcast_to(
                  [P, NCHUNK, H6])
              if 'abuild' not in skip:
                  nc.vector.tensor_tensor(out=ht[:], in0=iota6_bc,
                                          in1=dsth_bc,
                                          op=mybir.AluOpType.is_equal)
                  nc.vector.tensor_tensor(out=ht[:], in0=ht[:], in1=val_bc,
                                          op=mybir.AluOpType.mult)
                  nc.vector.tensor_tensor(out=lt[:], in0=iota8_bc,
                                          in1=dstl_bc,
                                          op=mybir.AluOpType.is_equal)
                  # at[p, c, nh*L8 + nl] = ht[p, c, nh] * lt[p, c, nl]
                  ht_ap, lt_ap, at_ap = ht[:], lt[:], at[:]
                  ht_bc = AP(ht_ap.tensor, ht_ap.offset,
                             [ht_ap.ap[0], ht_ap.ap[1], ht_ap.ap[2],
                              [0, L8]])
                  lt_bc = AP(lt_ap.tensor, lt_ap.offset,
                             [lt_ap.ap[0], lt_ap.ap[1], [0, H6],
                              lt_ap.ap[2]])
                  at4 = AP(at_ap.tensor, at_ap.offset,
                           [at_ap.ap[0], at_ap.ap[1], [L8, H6], [1, L8]])
                  nc.vector.tensor_tensor(out=at4, in0=ht_bc, in1=lt_bc,
                                          op=mybir.AluOpType.mult)

              psg = pspool.tile([F, GN], f32)
              if 'mm' in skip:
                  nc.vector.tensor_copy(psg[:, :TILE], at[:F, 0, :])
              else:
                  # tile-major emission: each tile's accumulation group
                  # (start at r=0, stop at r=R-1) closes before the next opens
                  for j in range(GROUP_TILES):
                      for r in range(R):
                          q = r * GROUP_TILES + j
                          nc.tensor.matmul(
                              out=psg[:, j * TILE:(j + 1) * TILE],
                              lhsT=gt[:, q, :],
                              rhs=at[:, q, :],
                              start=(r == 0),
                              stop=(r == R - 1),
                          )

              sup = spool.tile([F, GN], f32)
              nc.vector.tensor_add(sup[:], psg[:],
                                   f0_sb[:, g * GN:(g + 1) * GN])

              ps2 = ps2pool.tile([F, GN], f32)
              nc.tensor.matmul(ps2[:], lhsT=w_sb[:], rhs=sup[:],
                               start=True, stop=True)

              ot = opool.tile([F, GN], f32)
              nc.scalar.activation(ot[:], ps2[:],
                                   mybir.ActivationFunctionType.Relu)
              nc.sync.dma_start(out_d[:, g * GN:(g + 1) * GN], ot[:])

    return nc


def kernel(features, features0, edge_src, edge_dst, edge_vals, W):
    _import_concourse()
    from concourse.bass_utils import run_bass_kernel_spmd

    features = np.asarray(features, np.float32)
    features0 = np.asarray(features0, np.float32)
    edge_src = np.asarray(edge_src, np.int32)
    edge_dst = np.asarray(edge_dst, np.int32)
    edge_vals = np.asarray(edge_vals, np.float32)
    W = np.asarray(W, np.float32)

    in_maps, T, node_cols = _prep(
        features, features0, edge_src, edge_dst, edge_vals, W)
    nc = _build(T)
    nc.finalize()
    res = run_bass_kernel_spmd(nc, in_maps, list(range(N_CORES)))
    outs = []
    for i in range(N_CORES):
        outT = res.results[i]["outT"]            # [F, T*TILE]
        outs.append(outT[:, node_cols[i]].T)
    return np.ascontiguousarray(np.concatenate(outs, axis=0), dtype=np.float32)

